# revision 1
# baseline (speedup 1.0000x reference)
"""DeepPoly SPU transformer — Trainium2 Bass kernel (fp16 I/O edition).

Elementwise over N=16777216; sharded across 8 NeuronCores (2M elems each,
viewed as [nt x 128 x fd] fp16).  All wire traffic is fp16 (24MB/core round
trip vs 48MB in f32), which halves the DMA floor; compute is spread across
ACT / DVE / Pool so every engine stays near the DMA roofline.  The three
input streams are packed into one DRAM tensor (and the three outputs into
another) so each chunk needs a single input DMA + a single output DMA.
The chunk loop is emitted as an explicit 4-deep software pipeline
(DMA-prefetch / heads / mids / tails) so every cross-engine dependency is
at least one stage old when the consuming engine reaches it.

Math (per element; Z = sqrt(0.5), spu(t) = t^2-0.5 for t>=0 else sigmoid(-t)-1):
  out = relu(x)^2 - sigmoid(min(x,0))                      [exact identity]
  nu  = max(relu(u)^2, sigmoid(-l)-0.5) + ([l>=0] - 0.5)   [+0.5-space fold]
  nl  = Pk + [u<=0]*num2P  on device (+0.5 space; host epilogue adds -0.5),
        where
        num2P = max(relu(u)^2, sigmoid(-l)-0.5)
        P  = g2m*(l - g2m/4),  g2m = max(u+l, 2Z*[l<0])   (tangent parabola)
        Pk = max(P, -BIG*([u>=Z] or [l>=0]))   (clamps P to 0 in cases A/D,
             where g2m = 2Z makes P = 2Z*l - 0.5 < 0 exactly)

Case boundaries (u vs Z is a genuine jump in the reference) are pinned to the
f32 side during host-side fp16 conversion, so fp16 rounding never flips an
element across a discontinuity.  Validated in numpy emulation:
relmax_vs_scale ~ 1.1e-3 on all three outputs (tolerance 2e-2).
"""

import numpy as np

import concourse.bass as bass
import concourse.bacc as bacc
import concourse.mybir as mybir
from concourse.tile import TileContext
from concourse.bass_utils import run_bass_kernel_spmd

_N = 16777216
_NCORES = 8
_P = 128
_FDT = _N // _NCORES // _P  # 16384 free elems per partition per core

_Z32 = np.float32(np.sqrt(0.5))       # reference threshold (f32)
_Z16 = float(np.float16(np.sqrt(0.5)))
_TWO_Z = float(np.float16(2 * np.sqrt(0.5)))
_BIG = 1000.0

_AF = mybir.ActivationFunctionType
_OP = mybir.AluOpType
_F16 = mybir.dt.float16


def _build_nc(fd=2048, io_bufs=2, out_bufs=3, tmp_bufs=2, deep_bufs=3, ramp="end",
              slh_eng="dve", q_eng="dve", nl_eng="act", g2m_eng="pool",
              num2_eng="pool", bm_eng="pool", g2_eng="pool", o_eng="dve",
              mask_eng="dve", ru_alt=0, nu_alt=0, in_q="sp", out_q="sp", emit_order="tail_first", nl_alt=0, wp_alt=0, mbh_alt=0, n_lat=0, lat_eng=0, q_alt=4, q_ph=0, rx_alt=0, slh_alt=0, eager_tail=False, tail_act=0, q_set=None, t2_bufs=2, mbbn_bufs=2, n2_bufs=2, s2_dve=0, s2_first=False, fdt=_FDT):
    nc = bacc.Bacc(trn_type="TRN2", debug=False, num_devices=_NCORES)
    nt = fdt // fd
    # packed streams: [l | u | x] along the free dim, one DMA per chunk
    t_in = nc.dram_tensor("pin", [nt, _P, 3 * fd], _F16, kind="ExternalInput")
    t_out = nc.dram_tensor("pout", [nt, _P, 3 * fd], _F16, kind="ExternalOutput")

    def eng(name):
        return {"dve": nc.vector, "pool": nc.gpsimd}[name]

    in_dma = {"sp": nc.sync.dma_start, "act": nc.scalar.dma_start}[in_q]
    out_dma = {"sp": nc.sync.dma_start, "act": nc.scalar.dma_start}[out_q]

    with TileContext(nc) as tc:
        with tc.tile_pool(name="io", bufs=io_bufs) as iop, \
             tc.tile_pool(name="ot", bufs=out_bufs) as otp, \
             tc.tile_pool(name="tmp", bufs=tmp_bufs) as tp:

            if ramp == "both":
                chunks = [(0, c, fd // 4) for c in range(0, fd, fd // 4)]
                chunks += [(i, 0, fd) for i in range(1, nt - 1)]
                chunks += [(nt - 1, c, fd // 2) for c in range(0, fd, fd // 2)]
            elif ramp == "start":
                chunks = [(0, c, fd // 2) for c in range(0, fd, fd // 2)]
                chunks += [(i, 0, fd) for i in range(1, nt)]
            elif ramp == "end":
                chunks = [(i, 0, fd) for i in range(nt - 1)]
                chunks += [(nt - 1, c, fd // 2) for c in range(0, fd, fd // 2)]
            elif ramp == "split":
                # one half of the last tile leads (cheap fill), one drains
                chunks = [(nt - 1, 0, fd // 2)]
                chunks += [(i, 0, fd) for i in range(nt - 1)]
                chunks += [(nt - 1, fd // 2, fd // 2)]
            elif ramp == "split4":
                # quarters of the last tile: three lead, one drains
                chunks = [(nt - 1, c, fd // 4) for c in range(0, 3 * fd // 4, fd // 4)]
                chunks += [(i, 0, fd) for i in range(nt - 1)]
                chunks += [(nt - 1, 3 * fd // 4, fd // 4)]
            elif ramp == "end4":
                chunks = [(i, 0, fd) for i in range(nt - 1)]
                chunks += [(nt - 1, c, fd // 4) for c in range(0, fd, fd // 4)]
            elif ramp == "end2x":
                chunks = [(i, 0, fd) for i in range(nt - 2)]
                chunks += [(i, c, fd // 2) for i in (nt - 2, nt - 1)
                           for c in range(0, fd, fd // 2)]
            else:
                chunks = [(i, 0, fd) for i in range(nt)]

            # --- explicit 3-stage software pipeline -------------------
            # Pool fp16 supports tt add/sub/mult + any ts (NO tt max/min),
            # so Pool owns the linear combines (s2, nu, o) and DVE owns all
            # max/min. Stage skew keeps cross-engine deps >= 1 stage old.
            st = [None] * len(chunks)

            def SDMA(ci):
                i, c0, fdc = chunks[ci]
                with tc.high_priority():
                    it = iop.tile([_P, 3 * fdc], _F16, tag="in")
                    if fdc == fd:
                        in_dma(out=it[:], in_=t_in[i, :, 0:3 * fd])
                    else:  # partial chunk: per-stream slices of the packed row
                        for s in range(3):
                            in_dma(out=it[:, s * fdc:(s + 1) * fdc],
                                   in_=t_in[i, :, s * fd + c0:s * fd + c0 + fdc])
                st[ci] = dict(it=it)

            def S0(ci):
                i, c0, fdc = chunks[ci]
                it = st[ci]["it"]
                l = it[:, 0:fdc]
                u = it[:, fdc:2 * fdc]
                sl = tp.tile([_P, fdc], _F16, tag="sl", bufs=3)
                nc.scalar.activation(sl[:], l, _AF.Sigmoid, scale=-1.0)
                if s2_first:
                    s2 = tp.tile([_P, fdc], _F16, tag="s2", bufs=3)
                    lat = ci >= len(chunks) - max(n_lat, lat_eng)
                    s2d = lat or (s2_dve and (ci % s2_dve == 0))
                    (nc.vector if s2d else nc.gpsimd).tensor_tensor(s2[:], u, l, _OP.add)
                T2 = tp.tile([_P, fdc], _F16, tag="T2", bufs=t2_bufs)
                nc.vector.tensor_scalar(T2[:], l, 0.0, _TWO_Z, _OP.is_lt, _OP.mult)
                mBBn = tp.tile([_P, fdc], _F16, tag="mBBn", bufs=mbbn_bufs)
                nc.vector.tensor_scalar(mBBn[:], l, 0.0, -_BIG, _OP.is_ge, _OP.mult)
                mZBn = tp.tile([_P, fdc], _F16, tag="mZBn", bufs=3)
                nc.vector.tensor_scalar(mZBn[:], u, _Z16, -_BIG, _OP.is_ge, _OP.mult)
                mBh = tp.tile([_P, fdc], _F16, tag="mBh", bufs=3)
                mbh_pool = mbh_alt and (ci % mbh_alt == 0)
                (nc.gpsimd if mbh_pool else nc.vector).tensor_scalar(
                    mBh[:], l, 0.0, -0.5, _OP.is_ge, _OP.add)
                mA = tp.tile([_P, fdc], _F16, tag="mA", bufs=3)
                nc.vector.tensor_scalar(mA[:], u, 0.0, None, _OP.is_le)
                nc.vector.tensor_tensor(mZBn[:], mZBn[:], mBBn[:], _OP.min)  # BMn
                if not s2_first:
                    s2 = tp.tile([_P, fdc], _F16, tag="s2", bufs=3)
                    lat = ci >= len(chunks) - max(n_lat, lat_eng)
                    s2d = lat or (s2_dve and (ci % s2_dve == 0))
                    (nc.vector if s2d else nc.gpsimd).tensor_tensor(s2[:], u, l, _OP.add)
                st[ci].update(sl=sl, T2=T2, s2=s2, mZBn=mZBn,
                              mBh=mBh, mA=mA)

            def S1(ci):
                i, c0, fdc = chunks[ci]
                d = st[ci]
                it = d["it"]
                u = it[:, fdc:2 * fdc]
                x = it[:, 2 * fdc:3 * fdc]
                s2, sl = d["s2"], d["sl"]
                nc.vector.tensor_tensor(s2[:], s2[:], d["T2"][:], _OP.max)  # g2m
                tail_a = ci >= len(chunks) - tail_act
                ru = tp.tile([_P, fdc], _F16, tag="ru", bufs=3)
                if tail_a or (ru_alt and (ci % ru_alt == 0)):
                    nc.scalar.activation(ru[:], u, _AF.Relu)
                else:
                    nc.vector.tensor_scalar(ru[:], u, 0.0, None, _OP.max)
                nc.scalar.activation(ru[:], ru[:], _AF.Square)            # relu(u)^2
                if tail_a or (slh_alt and (ci % slh_alt == 0)):
                    nc.scalar.activation(sl[:], sl[:], _AF.Copy, bias=-0.5)
                else:
                    nc.vector.tensor_scalar(sl[:], sl[:], -0.5, None, _OP.add)  # slh
                rn = tp.tile([_P, fdc], _F16, tag="rn", bufs=3)
                nc.scalar.activation(rn[:], x, _AF.Relu, scale=-1.0)
                nc.scalar.activation(rn[:], rn[:], _AF.Sigmoid, scale=-1.0)
                rx = tp.tile([_P, fdc], _F16, tag="rx", bufs=3)
                if rx_alt and (ci % rx_alt == 0):
                    nc.vector.tensor_scalar(rx[:], x, 0.0, None, _OP.max)
                else:
                    nc.scalar.activation(rx[:], x, _AF.Relu)
                nc.scalar.activation(rx[:], rx[:], _AF.Square)            # relu(x)^2
                q = tp.tile([_P, fdc], _F16, tag="q", bufs=3)
                if (not tail_a) and ((q_set is not None and ci in q_set) or
                                     (q_set is None and q_alt and (ci % q_alt == q_ph))):
                    nc.vector.tensor_scalar(q[:], s2[:], -0.25, None, _OP.mult)
                else:
                    nc.scalar.activation(q[:], s2[:], _AF.Copy, scale=-0.25)
                l = it[:, 0:fdc]
                nc.vector.tensor_tensor(q[:], q[:], l, _OP.add)           # w
                d.update(q=q, ru=ru, rn=rn, rx=rx)

            def S2(ci):
                i, c0, fdc = chunks[ci]
                d = st[ci]
                ot = otp.tile([_P, 3 * fdc], _F16, tag="out")
                o_t = ot[:, 0:fdc]
                nl_t = ot[:, fdc:2 * fdc]
                nu_t = ot[:, 2 * fdc:3 * fdc]
                sl, q, mA, s2 = d["sl"], d["q"], d["mA"], d["s2"]
                n2 = tp.tile([_P, fdc], _F16, tag="n2", bufs=n2_bufs)
                nc.vector.tensor_tensor(n2[:], d["ru"][:], sl[:], _OP.max)
                nc.vector.tensor_tensor(q[:], s2[:], q[:], _OP.mult)      # P
                lat = ci >= len(chunks) - max(n_lat, lat_eng)
                (nc.vector if lat else nc.gpsimd).tensor_tensor(
                    nu_t, n2[:], d["mBh"][:], _OP.add)
                if eager_tail and fdc != fd:
                    out_dma(out=t_out[i, :, 2 * fd + c0:2 * fd + c0 + fdc],
                            in_=nu_t)
                nc.vector.tensor_tensor(q[:], q[:], d["mZBn"][:], _OP.max)  # Pk
                bm_pool = (wp_alt and (ci % wp_alt == 0)) or (s2_dve and (ci % s2_dve == 0))
                (nc.gpsimd if bm_pool else nc.vector).tensor_tensor(
                    mA[:], mA[:], n2[:], _OP.mult)                        # bmul
                (nc.vector if lat else nc.gpsimd).tensor_tensor(
                    o_t, d["rx"][:], d["rn"][:], _OP.subtract)
                if eager_tail and fdc != fd:
                    out_dma(out=t_out[i, :, c0:c0 + fdc], in_=o_t)
                # nl is emitted in +0.5 space (Pk + [u<=0]*num2P); the host
                # folds the -0.5 into its fp16->f32 conversion epilogue.
                nc.vector.tensor_tensor(nl_t, q[:], mA[:], _OP.add)       # badd

                if fdc == fd:
                    out_dma(out=t_out[i, :, 0:3 * fd], in_=ot[:])
                elif eager_tail:
                    out_dma(out=t_out[i, :, fd + c0:fd + c0 + fdc], in_=nl_t)
                else:
                    for s in range(3):
                        out_dma(out=t_out[i, :, s * fd + c0:s * fd + c0 + fdc],
                                in_=ot[:, s * fdc:(s + 1) * fdc])
                st[ci] = None

            n = len(chunks)
            order = {
                "dma_first": (lambda k: [(SDMA, k), (S0, k - 1), (S1, k - 2), (S2, k - 3)]),
                "tail_first": (lambda k: [(S2, k - 3), (S1, k - 2), (S0, k - 1), (SDMA, k)]),
                "mid": (lambda k: [(SDMA, k), (S2, k - 3), (S0, k - 1), (S1, k - 2)]),
                "o2": (lambda k: [(S1, k - 2), (S2, k - 3), (SDMA, k), (S0, k - 1)]),
                "o3": (lambda k: [(S2, k - 3), (SDMA, k), (S1, k - 2), (S0, k - 1)]),
                "o4": (lambda k: [(SDMA, k), (S1, k - 2), (S2, k - 3), (S0, k - 1)]),
                "o5": (lambda k: [(S0, k - 1), (SDMA, k), (S2, k - 3), (S1, k - 2)]),
                "o6": (lambda k: [(S1, k - 2), (S0, k - 1), (S2, k - 3), (SDMA, k)]),
                "o7": (lambda k: [(S2, k - 3), (S0, k - 1), (S1, k - 2), (SDMA, k)]),
                "o8": (lambda k: [(S0, k - 1), (S1, k - 2), (S2, k - 3), (SDMA, k)]),
            }[emit_order]
            n_skew = n - n_lat
            for k in range(n_skew + 3):
                for fn, ci in order(k):
                    if fn is SDMA:
                        if 0 <= ci < n:        # prefetch ALL chunks early
                            fn(ci)
                    elif 0 <= ci < n_skew:
                        fn(ci)
            for ci in range(n_skew, n):        # final chunks: tight, unskewed
                S0(ci)
                S1(ci)
                S2(ci)

    nc.compile()
    return nc


_NC_CACHE = {}


def _get_nc(**kw):
    key = tuple(sorted(kw.items()))
    if key not in _NC_CACHE:
        _NC_CACHE[key] = _build_nc(**kw)
    return _NC_CACHE[key]


def _prep_inputs(x, lower_bounds, upper_bounds):
    """fp16 conversion with case-boundary pinning (see module docstring)."""
    F16 = np.float16
    x16 = x.astype(F16)
    l16 = lower_bounds.astype(F16)
    u16 = upper_bounds.astype(F16)
    # l<0 must stay strictly negative in fp16 (is_ge(-0,0) is true).
    l16 = np.where((lower_bounds < 0) & (l16 >= 0), F16(-6e-8), l16)
    # u>0 must stay strictly positive (case A/D selection uses u<=0).
    u16 = np.where((upper_bounds > 0) & (u16 <= 0), F16(6e-8), u16)
    # u vs Z: the reference jumps at u==Z; keep each element on its f32 side.
    z16 = F16(_Z16)
    below = np.nextafter(z16, F16(0))
    u16 = np.where((upper_bounds >= _Z32) & (u16 < z16), z16, u16)
    u16 = np.where((upper_bounds < _Z32) & (u16 >= z16), below, u16)
    return x16, l16, u16


def _run(x, lower_bounds, upper_bounds, trace=False, **build_kw):
    assert x.shape == (_N,) and x.dtype == np.float32
    nc = _get_nc(**build_kw)
    fd = build_kw.get("fd", 2048)
    nt = _FDT // fd
    x16, l16, u16 = _prep_inputs(x, lower_bounds, upper_bounds)
    shp = (_NCORES, nt, _P, fd)
    packed = np.empty((_NCORES, nt, _P, 3 * fd), dtype=np.float16)
    packed[..., 0:fd] = l16.reshape(shp)
    packed[..., fd:2 * fd] = u16.reshape(shp)
    packed[..., 2 * fd:3 * fd] = x16.reshape(shp)
    in_maps = [{"pin": packed[c]} for c in range(_NCORES)]
    res = run_bass_kernel_spmd(
        nc, in_maps, core_ids=list(range(_NCORES)), trace=trace
    )
    pout = np.stack([res.results[c]["pout"] for c in range(_NCORES)])
    out = np.ascontiguousarray(pout[..., 0:fd]).reshape(-1).astype(np.float32)
    nl = np.ascontiguousarray(pout[..., fd:2 * fd]).reshape(-1).astype(np.float32)
    nl -= 0.5  # device computes new_lower in +0.5 space
    nu = np.ascontiguousarray(pout[..., 2 * fd:3 * fd]).reshape(-1).astype(np.float32)
    return (out, nl, nu), res


def kernel(x, lower_bounds, upper_bounds):
    (out, nl, nu), _ = _run(x, lower_bounds, upper_bounds)
    return (out, nl, nu)



# revision 2
# speedup vs baseline: 1.4516x; 1.4516x over previous
"""DeepPoly SPU transformer — Trainium2 Bass kernel (custom-DVE edition).

Elementwise over N=16777216; sharded across 8 NeuronCores (2M elems each,
viewed as [nt x 128 x fd] fp16).  All wire traffic is fp16 (24MB/core round
trip); the three input streams are packed into one DRAM tensor (and the
three outputs into another) so each chunk needs one input DMA + one output
DMA.  The whole per-element DAG is collapsed into 3 fused custom-DVE ops +
2 ACT transcendentals + a small ACT/Pool tail, so every engine sits below
the DMA roofline (~8.7us per 128x2048 chunk) and the kernel is purely
DMA-bound.

Math (per element; Z = sqrt(0.5), spu(t) = t^2-0.5 for t>=0 else
sigmoid(-t)-1).  Device emits nl/nu in doubled space (host applies the
constant affine epilogue  nl = 0.5*nlD - 0.5,  nu = 0.5*nuD):

  sld = tanh(-l/2) = 2*(sigmoid(-l) - 0.5)      [ACT; sign(sld) = -sign(l)]
  sgx = sigmoid(x)                              [ACT]
  out = relu(x)^2 - min(sgx, 0.5)               [ACT relu/square + Pool]
  P2  = 2*g2m*(l - g2m/4),  g2m = 2Z*max([l<0], (u+l)/(2Z))   [custom DVE:
        G1 = max([l<0], (u+l)/(2Z));  P2 = G1*(4Z*l - G1)]
  SD  = sld*[u<=0] - BIG*([u>=Z] + [sld<=0])    [custom DVE]
  nuD = max(2*relu(u)^2, sld) + (2*[sld<=0]-1)  [custom DVE]
  nlD = max(P2, SD)                             [DVE tt max]

Case boundaries (l vs 0, u vs 0, u vs Z) are pinned to the f32 side during
host-side fp16 conversion; l<0 is additionally kept <= -2.5e-7 so that
tanh(-l/2) cannot round to +-0 in fp16 (sld's sign carries [l<0] on
device).  Validated vs the f64 reference: relmax_vs_scale ~ 1e-3 on all
three outputs (tolerance 2e-2).
"""

import numpy as np

import concourse.bass as bass
import concourse.bacc as bacc
import concourse.mybir as mybir
from concourse.tile import TileContext
from concourse.bass_utils import run_bass_kernel_spmd
import concourse.dve_ops as dve_ops
from concourse.dve_spec import (
    Spec, Src0, Src1, C0, C1, Zero, One, maxx, minn, relu, sq, lower as _dve_lower,
    _has_src1,
)
from concourse.dve_uop import DveOpSpec

_N = 16777216
_NCORES = 8
_P = 128
_FDT = _N // _NCORES // _P  # 16384 free elems per partition per core

_Z32 = np.float32(np.sqrt(0.5))
_Z16 = float(np.float16(np.sqrt(0.5)))
_INV2Z = float(np.float32(1.0 / (2.0 * float(_Z32))))
_FOURZ = float(np.float32(4.0 * float(_Z32)))
_BIG = 1000.0

_AF = mybir.ActivationFunctionType
_OP = mybir.AluOpType
_F16 = mybir.dt.float16


# --------------------------------------------------------------------------
# custom DVE ops (registered once per process via the documented extension
# point in concourse.dve_ops; names are namespaced to this kernel)
# --------------------------------------------------------------------------

def _register(name, spec):
    if name in dve_ops._SUB_OPCODE_FOR_NAME:
        for op in dve_ops.OPS:
            if op.name == name:
                return op
    row = max(dve_ops._SUB_OPCODE_FOR_NAME.values()) + 1
    assert row < 0x20, "custom-DVE opcode rows exhausted"
    shas = {}
    for ver in ("v3", "v4"):
        u = _dve_lower(spec, ver=ver)
        shas[ver] = DveOpSpec(
            name=name, opcode=row, uops=u, rd1_en=_has_src1(spec)
        ).sha(ver)
    op = dve_ops.DveOp(name, spec, subdim=False, uops_sha=shas)
    dve_ops.OPS.append(op)
    dve_ops._SUB_OPCODE_FOR_NAME[name] = row
    dve_ops.CUSTOM_DVE_SPECS[name] = spec
    return op


def _ref_p2(in0, in1, c0, c1, c2):
    l = in0.astype(np.float32)
    u = in1.astype(np.float32)
    s2s = (l + u) * np.float32(c0)
    g1 = np.maximum((l < 0).astype(np.float32), s2s)
    return g1 * (l * np.float32(c1) - g1)


_g1 = maxx(Src0 < Zero, (Src0 + Src1) * C0)
_SPEC_P2 = Spec(body=_g1 * (Src0 * C1 - _g1), reference=_ref_p2)


def _ref_sd(in0, in1, c0, c1, c2):
    u = in0.astype(np.float32)
    sld = in1.astype(np.float32)
    m1 = (u <= 0).astype(np.float32)
    m2 = (u >= np.float32(c0)).astype(np.float32)
    m3 = (sld <= 0).astype(np.float32)
    return sld * m1 + (m2 + m3) * np.float32(c1)


_SPEC_SD = Spec(
    body=Src1 * (Src0 <= Zero) + ((Src0 >= C0) + (Src1 <= Zero)) * C1,
    reference=_ref_sd,
)


def _ref_nu(in0, in1, c0, c1, c2):
    u = in0.astype(np.float32)
    sld = in1.astype(np.float32)
    r2 = np.maximum(u, 0) ** 2
    c = (sld <= 0).astype(np.float32)
    return np.maximum(r2 + r2, sld) + (c + c - 1.0)


_r2 = sq(relu(Src0))
_c = (Src1 <= Zero)
_SPEC_NU = Spec(body=maxx(_r2 + _r2, Src1) + (_c + _c - One), reference=_ref_nu)


def _ref_out(in0, in1, c0, c1, c2):
    x = in0.astype(np.float32)
    sgx = in1.astype(np.float32)
    return np.maximum(x, 0) ** 2 - np.minimum(sgx, np.float32(c0))


_SPEC_OUT = Spec(body=sq(relu(Src0)) - minn(Src1, C0), reference=_ref_out)

_OP_P2 = _register("SPU_P2_ANT", _SPEC_P2)
_OP_SD = _register("SPU_SD_ANT", _SPEC_SD)
_OP_NU = _register("SPU_NU_ANT", _SPEC_NU)
_OP_OUT = _register("SPU_OUT_ANT", _SPEC_OUT)


# --------------------------------------------------------------------------
# kernel build
# --------------------------------------------------------------------------

def _build_nc(fd=2048, io_bufs=3, out_bufs=3, tmp_bufs=3, ramp="end4",
              n_dve_out=0, rx_dve=0, fdt=_FDT):
    """n_dve_out: chunks whose `out` stream is computed by the fused DVE op
    instead of the ACT/Pool tail (load-balance knob).  rx_dve: chunks whose
    relu(x) runs as a DVE tensor_scalar instead of ACT."""
    nc = bacc.Bacc(trn_type="TRN2", debug=False, num_devices=_NCORES)
    nt = fdt // fd
    t_in = nc.dram_tensor("pin", [nt, _P, 3 * fd], _F16, kind="ExternalInput")
    t_out = nc.dram_tensor("pout", [nt, _P, 3 * fd], _F16, kind="ExternalOutput")

    with TileContext(nc) as tc:
        with tc.tile_pool(name="io", bufs=io_bufs) as iop, \
             tc.tile_pool(name="ot", bufs=out_bufs) as otp, \
             tc.tile_pool(name="tmp", bufs=tmp_bufs) as tp:

            if ramp == "end":
                chunks = [(i, 0, fd) for i in range(nt - 1)]
                chunks += [(nt - 1, c, fd // 2) for c in range(0, fd, fd // 2)]
            elif ramp == "end4":
                chunks = [(i, 0, fd) for i in range(nt - 1)]
                chunks += [(nt - 1, c, fd // 4) for c in range(0, fd, fd // 4)]
            elif ramp == "both":
                chunks = [(0, c, fd // 2) for c in range(0, fd, fd // 2)]
                chunks += [(i, 0, fd) for i in range(1, nt - 1)]
                chunks += [(nt - 1, c, fd // 4) for c in range(0, fd, fd // 4)]
            else:
                chunks = [(i, 0, fd) for i in range(nt)]

            n = len(chunks)
            st = [None] * n

            def SDMA(ci):
                i, c0, fdc = chunks[ci]
                with tc.high_priority():
                    it = iop.tile([_P, 3 * fdc], _F16, tag="in")
                    if fdc == fd:
                        nc.sync.dma_start(out=it[:], in_=t_in[i, :, 0:3 * fd])
                    else:
                        for s in range(3):
                            nc.sync.dma_start(
                                out=it[:, s * fdc:(s + 1) * fdc],
                                in_=t_in[i, :, s * fd + c0:s * fd + c0 + fdc])
                st[ci] = dict(it=it)

            def S0(ci):
                i, c0, fdc = chunks[ci]
                it = st[ci]["it"]
                l = it[:, 0:fdc]
                x = it[:, 2 * fdc:3 * fdc]
                sld = tp.tile([_P, fdc], _F16, tag="sld")
                sgx = tp.tile([_P, fdc], _F16, tag="sgx")
                nc.scalar.activation(sld[:], l, _AF.Tanh, scale=-0.5)
                nc.scalar.activation(sgx[:], x, _AF.Sigmoid)
                dve_out = ci < n_dve_out
                if not dve_out:
                    rx2 = tp.tile([_P, fdc], _F16, tag="rx2")
                    if rx_dve and (ci % rx_dve == 0):
                        rx = tp.tile([_P, fdc], _F16, tag="rx")
                        nc.vector.tensor_scalar(rx[:], x, 0.0, None, _OP.max)
                        nc.scalar.activation(rx2[:], rx[:], _AF.Square)
                    else:
                        nc.scalar.activation(rx2[:], x, _AF.Relu)
                        nc.scalar.activation(rx2[:], rx2[:], _AF.Square)
                    st[ci].update(rx2=rx2)
                st[ci].update(sld=sld, sgx=sgx, dve_out=dve_out)

            def S1(ci):
                i, c0, fdc = chunks[ci]
                d = st[ci]
                it = d["it"]
                l3 = it[:, 0:fdc].unsqueeze(1)
                u3 = it[:, fdc:2 * fdc].unsqueeze(1)
                sld3 = d["sld"][:].unsqueeze(1)
                ot = otp.tile([_P, 3 * fdc], _F16, tag="out")
                p2 = tp.tile([_P, fdc], _F16, tag="p2")
                sd = tp.tile([_P, fdc], _F16, tag="sd")
                nc.vector._custom_dve(_OP_P2, out=p2[:].unsqueeze(1), in0=l3,
                                      in1=u3, s0=_INV2Z, s1=_FOURZ)
                nc.vector._custom_dve(_OP_SD, out=sd[:].unsqueeze(1), in0=u3,
                                      in1=sld3, s0=_Z16, s1=-_BIG)
                nc.vector._custom_dve(_OP_NU, out=ot[:, 2 * fdc:3 * fdc].unsqueeze(1),
                                      in0=u3, in1=sld3)
                if d["dve_out"]:
                    x3 = it[:, 2 * fdc:3 * fdc].unsqueeze(1)
                    nc.vector._custom_dve(_OP_OUT, out=ot[:, 0:fdc].unsqueeze(1),
                                          in0=x3, in1=d["sgx"][:].unsqueeze(1),
                                          s0=0.5)
                else:
                    sg = tp.tile([_P, fdc], _F16, tag="sg")
                    nc.gpsimd.tensor_scalar(sg[:], d["sgx"][:], 0.5, None, _OP.min)
                    st[ci].update(sg=sg)
                st[ci].update(ot=ot, p2=p2, sd=sd)

            def S2(ci):
                i, c0, fdc = chunks[ci]
                d = st[ci]
                ot = d["ot"]
                if not d["dve_out"]:
                    nc.gpsimd.tensor_tensor(ot[:, 0:fdc], d["rx2"][:], d["sg"][:],
                                            _OP.subtract)
                nc.vector.tensor_tensor(ot[:, fdc:2 * fdc], d["p2"][:], d["sd"][:],
                                        _OP.max)
                if fdc == fd:
                    nc.sync.dma_start(out=t_out[i, :, 0:3 * fd], in_=ot[:])
                else:
                    for s in range(3):
                        nc.sync.dma_start(
                            out=t_out[i, :, s * fd + c0:s * fd + c0 + fdc],
                            in_=ot[:, s * fdc:(s + 1) * fdc])
                st[ci] = None

            for k in range(n + 3):
                if k < n:
                    SDMA(k)
                if 0 <= k - 1 < n:
                    S0(k - 1)
                if 0 <= k - 2 < n:
                    S1(k - 2)
                if 0 <= k - 3 < n:
                    S2(k - 3)

    nc.compile()
    return nc


_NC_CACHE = {}


def _get_nc(**kw):
    key = tuple(sorted(kw.items()))
    if key not in _NC_CACHE:
        _NC_CACHE[key] = _build_nc(**kw)
    return _NC_CACHE[key]


def _prep_inputs(x, lower_bounds, upper_bounds):
    """fp16 conversion with case-boundary pinning (see module docstring)."""
    F16 = np.float16
    x16 = x.astype(F16)
    l16 = lower_bounds.astype(F16)
    u16 = upper_bounds.astype(F16)
    # l<0 must stay strictly negative AND large enough that tanh(-l/2)
    # cannot round to zero in fp16 (sld's sign carries [l<0] on device).
    l16 = np.where((lower_bounds < 0) & (l16 >= -2.5e-7), F16(-2.5e-7), l16)
    # u>0 must stay strictly positive (case selection uses u<=0).
    u16 = np.where((upper_bounds > 0) & (u16 <= 0), F16(6e-8), u16)
    # u vs Z: the reference jumps at u==Z; keep each element on its f32 side.
    z16 = F16(_Z16)
    below = np.nextafter(z16, F16(0))
    u16 = np.where((upper_bounds >= _Z32) & (u16 < z16), z16, u16)
    u16 = np.where((upper_bounds < _Z32) & (u16 >= z16), below, u16)
    return x16, l16, u16


def _run(x, lower_bounds, upper_bounds, trace=False, **build_kw):
    assert x.shape == (_N,) and x.dtype == np.float32
    nc = _get_nc(**build_kw)
    fd = build_kw.get("fd", 2048)
    nt = _FDT // fd
    x16, l16, u16 = _prep_inputs(x, lower_bounds, upper_bounds)
    shp = (_NCORES, nt, _P, fd)
    packed = np.empty((_NCORES, nt, _P, 3 * fd), dtype=np.float16)
    packed[..., 0:fd] = l16.reshape(shp)
    packed[..., fd:2 * fd] = u16.reshape(shp)
    packed[..., 2 * fd:3 * fd] = x16.reshape(shp)
    in_maps = [{"pin": packed[c]} for c in range(_NCORES)]
    res = run_bass_kernel_spmd(
        nc, in_maps, core_ids=list(range(_NCORES)), trace=trace
    )
    pout = np.stack([res.results[c]["pout"] for c in range(_NCORES)])
    out = np.ascontiguousarray(pout[..., 0:fd]).reshape(-1).astype(np.float32)
    nl = np.ascontiguousarray(pout[..., fd:2 * fd]).reshape(-1).astype(np.float32)
    nl = nl * 0.5 - 0.5  # device emits nl in doubled +0.5 space
    nu = np.ascontiguousarray(pout[..., 2 * fd:3 * fd]).reshape(-1).astype(np.float32)
    nu = nu * 0.5  # device emits nu in doubled space
    return (out, nl, nu), res


def kernel(x, lower_bounds, upper_bounds):
    (out, nl, nu), _ = _run(x, lower_bounds, upper_bounds)
    return (out, nl, nu)


# revision 14
# speedup vs baseline: 1.5300x; 1.0540x over previous
"""DeepPoly SPU transformer — Trainium2 Bass kernel (custom-DVE edition).

Elementwise over N=16777216; sharded across 8 NeuronCores (2M elems each,
viewed as [nt x 128 x fd] fp16).  All wire traffic is fp16 (24MB/core round
trip); the three input streams are packed into one DRAM tensor (and the
three outputs into another) so each chunk needs one input DMA + one output
DMA.  The whole per-element DAG is collapsed into 3 fused custom-DVE ops +
2 ACT transcendentals + a small ACT/Pool tail, so every engine sits below
the DMA roofline (~8.7us per 128x2048 chunk) and the kernel is purely
DMA-bound.

Math (per element; Z = sqrt(0.5), spu(t) = t^2-0.5 for t>=0 else
sigmoid(-t)-1).  Device emits nl/nu in doubled space (host applies the
constant affine epilogue  nl = 0.5*nlD - 0.5,  nu = 0.5*nuD):

  sld = tanh(-l/2) = 2*(sigmoid(-l) - 0.5)      [ACT; sign(sld) = -sign(l)]
  sgx = sigmoid(x)                              [ACT]
  out = relu(x)^2 - min(sgx, 0.5)               [ACT relu/square + Pool]
  P2  = 2*g2m*(l - g2m/4),  g2m = 2Z*max([l<0], (u+l)/(2Z))   [custom DVE:
        G1 = max([l<0], (u+l)/(2Z));  P2 = G1*(4Z*l - G1)]
  SD  = sld*[u<=0] - BIG*([u>=Z] + [sld<=0])    [custom DVE]
  nuD = max(2*relu(u)^2, sld) + (2*[sld<=0]-1)  [custom DVE]
  nlD = max(P2, SD)                             [DVE tt max]

Case boundaries (l vs 0, u vs 0, u vs Z) are pinned to the f32 side during
host-side fp16 conversion; l<0 is additionally kept <= -2.5e-7 so that
tanh(-l/2) cannot round to +-0 in fp16 (sld's sign carries [l<0] on
device).  Validated vs the f64 reference: relmax_vs_scale ~ 1e-3 on all
three outputs (tolerance 2e-2).
"""

import numpy as np

import concourse.bass as bass
import concourse.bacc as bacc
import concourse.mybir as mybir
from concourse.tile import TileContext
from concourse.bass_utils import run_bass_kernel_spmd
import concourse.dve_ops as dve_ops
from concourse.dve_spec import (
    Spec, Src0, Src1, C0, C1, Zero, One, maxx, minn, relu, sq, lower as _dve_lower,
    _has_src1,
)
from concourse.dve_uop import DveOpSpec

_N = 16777216
_NCORES = 8
_P = 128
_FDT = _N // _NCORES // _P  # 16384 free elems per partition per core

_Z32 = np.float32(np.sqrt(0.5))
_Z16 = float(np.float16(np.sqrt(0.5)))
_INV2Z = float(np.float32(1.0 / (2.0 * float(_Z32))))
_FOURZ = float(np.float32(4.0 * float(_Z32)))
_BIG = 1000.0

_AF = mybir.ActivationFunctionType
_OP = mybir.AluOpType
_F16 = mybir.dt.float16


# --------------------------------------------------------------------------
# custom DVE ops (registered once per process via the documented extension
# point in concourse.dve_ops; names are namespaced to this kernel)
# --------------------------------------------------------------------------

def _register(name, spec):
    if name in dve_ops._SUB_OPCODE_FOR_NAME:
        for op in dve_ops.OPS:
            if op.name == name:
                return op
    row = max(dve_ops._SUB_OPCODE_FOR_NAME.values()) + 1
    assert row < 0x20, "custom-DVE opcode rows exhausted"
    shas = {}
    for ver in ("v3", "v4"):
        u = _dve_lower(spec, ver=ver)
        shas[ver] = DveOpSpec(
            name=name, opcode=row, uops=u, rd1_en=_has_src1(spec)
        ).sha(ver)
    op = dve_ops.DveOp(name, spec, subdim=False, uops_sha=shas)
    dve_ops.OPS.append(op)
    dve_ops._SUB_OPCODE_FOR_NAME[name] = row
    dve_ops.CUSTOM_DVE_SPECS[name] = spec
    return op


def _ref_p2(in0, in1, c0, c1, c2):
    l = in0.astype(np.float32)
    u = in1.astype(np.float32)
    s2s = (l + u) * np.float32(c0)
    g1 = np.maximum((l < 0).astype(np.float32), s2s)
    return g1 * (l * np.float32(c1) - g1)


_g1 = maxx(Src0 < Zero, (Src0 + Src1) * C0)
_SPEC_P2 = Spec(body=_g1 * (Src0 * C1 - _g1), reference=_ref_p2)


def _ref_sd(in0, in1, c0, c1, c2):
    u = in0.astype(np.float32)
    sld = in1.astype(np.float32)
    m1 = (u <= 0).astype(np.float32)
    m2 = (u >= np.float32(c0)).astype(np.float32)
    m3 = (sld <= 0).astype(np.float32)
    return sld * m1 + (m2 + m3) * np.float32(c1)


_SPEC_SD = Spec(
    body=Src1 * (Src0 <= Zero) + ((Src0 >= C0) + (Src1 <= Zero)) * C1,
    reference=_ref_sd,
)


def _ref_nu(in0, in1, c0, c1, c2):
    u = in0.astype(np.float32)
    sld = in1.astype(np.float32)
    r2 = np.maximum(u, 0) ** 2
    c = (sld <= 0).astype(np.float32)
    return np.maximum(r2 + r2, sld) + (c + c - 1.0)


_r2 = sq(relu(Src0))
_c = (Src1 <= Zero)
_SPEC_NU = Spec(body=maxx(_r2 + _r2, Src1) + (_c + _c - One), reference=_ref_nu)


def _ref_out(in0, in1, c0, c1, c2):
    x = in0.astype(np.float32)
    sgx = in1.astype(np.float32)
    return np.maximum(x, 0) ** 2 - np.minimum(sgx, np.float32(c0))


_SPEC_OUT = Spec(body=sq(relu(Src0)) - minn(Src1, C0), reference=_ref_out)

_OP_P2 = _register("SPU_P2_ANT", _SPEC_P2)
_OP_SD = _register("SPU_SD_ANT", _SPEC_SD)
_OP_NU = _register("SPU_NU_ANT", _SPEC_NU)
_OP_OUT = _register("SPU_OUT_ANT", _SPEC_OUT)


# --------------------------------------------------------------------------
# kernel build
# --------------------------------------------------------------------------

def _build_nc(fd=2048, io_bufs=4, out_bufs=4, tmp_bufs=3, ramp="s24e2",
              dve_head=3, dve_tail=2, rx_dve=0, in_q="sp", out_q="sp",
              emit_order="dma_first", skew=3, hp_out=0, fdt=_FDT):
    """dve_head/dve_tail: how many chunks at each end of the pipeline route
    their `out` stream through the fused DVE op (low latency) instead of the
    ACT/Pool tail (better steady-state balance).  rx_dve: chunks whose
    relu(x) runs as a DVE tensor_scalar instead of ACT."""
    nc = bacc.Bacc(trn_type="TRN2", debug=False, num_devices=_NCORES)
    nt = fdt // fd

    def _q(name):
        return {"sp": nc.sync, "act": nc.scalar, "dve": nc.vector,
                "pool": nc.gpsimd}[name]

    t_in = nc.dram_tensor("pin", [nt, _P, 3, fd], _F16, kind="ExternalInput")
    t_out = nc.dram_tensor("pout", [nt, _P, 3, fd], _F16, kind="ExternalOutput")

    with TileContext(nc) as tc:
        with tc.tile_pool(name="io", bufs=io_bufs) as iop, \
             tc.tile_pool(name="ot", bufs=out_bufs) as otp, \
             tc.tile_pool(name="tmp", bufs=tmp_bufs) as tp:

            if ramp == "end":
                chunks = [(i, 0, fd) for i in range(nt - 1)]
                chunks += [(nt - 1, c, fd // 2) for c in range(0, fd, fd // 2)]
            elif ramp == "end4":
                chunks = [(i, 0, fd) for i in range(nt - 1)]
                chunks += [(nt - 1, c, fd // 4) for c in range(0, fd, fd // 4)]
            elif ramp == "both":
                chunks = [(0, c, fd // 2) for c in range(0, fd, fd // 2)]
                chunks += [(i, 0, fd) for i in range(1, nt - 1)]
                chunks += [(nt - 1, c, fd // 4) for c in range(0, fd, fd // 4)]
            elif ramp == "s4e2":
                chunks = [(0, c, fd // 4) for c in range(0, fd, fd // 4)]
                chunks += [(i, 0, fd) for i in range(1, nt - 1)]
                chunks += [(nt - 1, c, fd // 2) for c in range(0, fd, fd // 2)]
            elif ramp == "s4e4":
                chunks = [(0, c, fd // 4) for c in range(0, fd, fd // 4)]
                chunks += [(i, 0, fd) for i in range(1, nt - 1)]
                chunks += [(nt - 1, c, fd // 4) for c in range(0, fd, fd // 4)]
            elif ramp == "s2e2":
                chunks = [(0, c, fd // 2) for c in range(0, fd, fd // 2)]
                chunks += [(i, 0, fd) for i in range(1, nt - 1)]
                chunks += [(nt - 1, c, fd // 2) for c in range(0, fd, fd // 2)]
            elif ramp == "s24e2":
                chunks = [(0, 0, fd // 4), (0, fd // 4, fd // 4), (0, fd // 2, fd // 2)]
                chunks += [(i, 0, fd) for i in range(1, nt - 1)]
                chunks += [(nt - 1, c, fd // 2) for c in range(0, fd, fd // 2)]
            elif ramp == "s24e24":
                chunks = [(0, 0, fd // 4), (0, fd // 4, fd // 4), (0, fd // 2, fd // 2)]
                chunks += [(i, 0, fd) for i in range(1, nt - 1)]
                chunks += [(nt - 1, 0, fd // 2), (nt - 1, fd // 2, fd // 4),
                           (nt - 1, 3 * fd // 4, fd // 4)]
            elif ramp == "s24":
                chunks = [(0, 0, fd // 4), (0, fd // 4, fd // 4), (0, fd // 2, fd // 2)]
                chunks += [(i, 0, fd) for i in range(1, nt)]
            elif ramp == "s8e2":
                chunks = [(0, c, fd // 8) for c in range(0, fd, fd // 8)]
                chunks += [(i, 0, fd) for i in range(1, nt - 1)]
                chunks += [(nt - 1, c, fd // 2) for c in range(0, fd, fd // 2)]
            else:
                chunks = [(i, 0, fd) for i in range(nt)]

            n = len(chunks)
            st = [None] * n

            def SDMA(ci):
                i, c0, fdc = chunks[ci]
                with tc.high_priority():
                    it = iop.tile([_P, 3 * fdc], _F16, tag="in")
                    _q(in_q).dma_start(
                        out=it[:].rearrange("p (s f) -> p s f", s=3),
                        in_=t_in[i, :, :, c0:c0 + fdc])
                st[ci] = dict(it=it)

            def S0(ci):
                i, c0, fdc = chunks[ci]
                it = st[ci]["it"]
                l = it[:, 0:fdc]
                x = it[:, 2 * fdc:3 * fdc]
                sld = tp.tile([_P, fdc], _F16, tag="sld")
                sgx = tp.tile([_P, fdc], _F16, tag="sgx")
                nc.scalar.activation(sld[:], l, _AF.Tanh, scale=-0.5)
                nc.scalar.activation(sgx[:], x, _AF.Sigmoid)
                dve_out = ci < dve_head or ci >= n - dve_tail
                if not dve_out:
                    rx2 = tp.tile([_P, fdc], _F16, tag="rx2")
                    if rx_dve and (ci % rx_dve == 0):
                        rx = tp.tile([_P, fdc], _F16, tag="rx")
                        nc.vector.tensor_scalar(rx[:], x, 0.0, None, _OP.max)
                        nc.scalar.activation(rx2[:], rx[:], _AF.Square)
                    else:
                        nc.scalar.activation(rx2[:], x, _AF.Relu)
                        nc.scalar.activation(rx2[:], rx2[:], _AF.Square)
                    st[ci].update(rx2=rx2)
                st[ci].update(sld=sld, sgx=sgx, dve_out=dve_out)

            def S1(ci):
                i, c0, fdc = chunks[ci]
                d = st[ci]
                it = d["it"]
                l3 = it[:, 0:fdc].unsqueeze(1)
                u3 = it[:, fdc:2 * fdc].unsqueeze(1)
                sld3 = d["sld"][:].unsqueeze(1)
                ot = otp.tile([_P, 3 * fdc], _F16, tag="out")
                p2 = tp.tile([_P, fdc], _F16, tag="p2")
                sd = tp.tile([_P, fdc], _F16, tag="sd")
                nc.vector._custom_dve(_OP_P2, out=p2[:].unsqueeze(1), in0=l3,
                                      in1=u3, s0=_INV2Z, s1=_FOURZ)
                nc.vector._custom_dve(_OP_SD, out=sd[:].unsqueeze(1), in0=u3,
                                      in1=sld3, s0=_Z16, s1=-_BIG)
                nc.vector._custom_dve(_OP_NU, out=ot[:, 2 * fdc:3 * fdc].unsqueeze(1),
                                      in0=u3, in1=sld3)
                if d["dve_out"]:
                    x3 = it[:, 2 * fdc:3 * fdc].unsqueeze(1)
                    nc.vector._custom_dve(_OP_OUT, out=ot[:, 0:fdc].unsqueeze(1),
                                          in0=x3, in1=d["sgx"][:].unsqueeze(1),
                                          s0=0.5)
                else:
                    sg = tp.tile([_P, fdc], _F16, tag="sg")
                    nc.gpsimd.tensor_scalar(sg[:], d["sgx"][:], 0.5, None, _OP.min)
                    st[ci].update(sg=sg)
                st[ci].update(ot=ot, p2=p2, sd=sd)

            def S2(ci):
                i, c0, fdc = chunks[ci]
                d = st[ci]
                ot = d["ot"]
                if not d["dve_out"]:
                    nc.gpsimd.tensor_tensor(ot[:, 0:fdc], d["rx2"][:], d["sg"][:],
                                            _OP.subtract)
                nc.vector.tensor_tensor(ot[:, fdc:2 * fdc], d["p2"][:], d["sd"][:],
                                        _OP.max)
                if hp_out:
                    with tc.high_priority():
                        _q(out_q).dma_start(
                            out=t_out[i, :, :, c0:c0 + fdc],
                            in_=ot[:].rearrange("p (s f) -> p s f", s=3))
                else:
                    _q(out_q).dma_start(
                        out=t_out[i, :, :, c0:c0 + fdc],
                        in_=ot[:].rearrange("p (s f) -> p s f", s=3))
                st[ci] = None

            if skew == 0:
                for ci in range(n):
                    SDMA(ci); S0(ci); S1(ci); S2(ci)
            elif skew == 1:
                for k in range(n + 1):
                    if k < n: SDMA(k)
                    if 0 <= k - 1 < n:
                        S0(k - 1); S1(k - 1); S2(k - 1)
            elif skew == 2:
                for k in range(n + 2):
                    if k < n: SDMA(k)
                    if 0 <= k - 1 < n: S0(k - 1)
                    if 0 <= k - 2 < n: S1(k - 2); S2(k - 2)
            else:
                order = {
                    "dma_first": lambda k: [(SDMA, k), (S0, k - 1), (S1, k - 2), (S2, k - 3)],
                    "s2_mid": lambda k: [(SDMA, k), (S0, k - 1), (S2, k - 3), (S1, k - 2)],
                }[emit_order]
                for k in range(n + 3):
                    for fn, ci in order(k):
                        if 0 <= ci < n:
                            fn(ci)

    nc.compile()
    return nc


_NC_CACHE = {}


def _get_nc(**kw):
    key = tuple(sorted(kw.items()))
    if key not in _NC_CACHE:
        _NC_CACHE[key] = _build_nc(**kw)
    return _NC_CACHE[key]


def _prep_inputs(x, lower_bounds, upper_bounds):
    """fp16 conversion with case-boundary pinning (see module docstring)."""
    F16 = np.float16
    x16 = x.astype(F16)
    l16 = lower_bounds.astype(F16)
    u16 = upper_bounds.astype(F16)
    # l<0 must stay strictly negative AND large enough that tanh(-l/2)
    # cannot round to zero in fp16 (sld's sign carries [l<0] on device).
    l16 = np.where((lower_bounds < 0) & (l16 >= -2.5e-7), F16(-2.5e-7), l16)
    # u>0 must stay strictly positive (case selection uses u<=0).
    u16 = np.where((upper_bounds > 0) & (u16 <= 0), F16(6e-8), u16)
    # u vs Z: the reference jumps at u==Z; keep each element on its f32 side.
    z16 = F16(_Z16)
    below = np.nextafter(z16, F16(0))
    u16 = np.where((upper_bounds >= _Z32) & (u16 < z16), z16, u16)
    u16 = np.where((upper_bounds < _Z32) & (u16 >= z16), below, u16)
    return x16, l16, u16


def _run(x, lower_bounds, upper_bounds, trace=False, **build_kw):
    assert x.shape == (_N,) and x.dtype == np.float32
    nc = _get_nc(**build_kw)
    fd = build_kw.get("fd", 2048)
    nt = _FDT // fd
    x16, l16, u16 = _prep_inputs(x, lower_bounds, upper_bounds)
    shp = (_NCORES, nt, _P, fd)
    packed = np.empty((_NCORES, nt, _P, 3, fd), dtype=np.float16)
    packed[..., 0, :] = l16.reshape(shp)
    packed[..., 1, :] = u16.reshape(shp)
    packed[..., 2, :] = x16.reshape(shp)
    in_maps = [{"pin": packed[c]} for c in range(_NCORES)]
    res = run_bass_kernel_spmd(
        nc, in_maps, core_ids=list(range(_NCORES)), trace=trace
    )
    pout = np.stack([res.results[c]["pout"] for c in range(_NCORES)])
    out = np.ascontiguousarray(pout[..., 0, :]).reshape(-1).astype(np.float32)
    nl = np.ascontiguousarray(pout[..., 1, :]).reshape(-1).astype(np.float32)
    nl = nl * 0.5 - 0.5  # device emits nl in doubled +0.5 space
    nu = np.ascontiguousarray(pout[..., 2, :]).reshape(-1).astype(np.float32)
    nu = nu * 0.5  # device emits nu in doubled space
    return (out, nl, nu), res


def kernel(x, lower_bounds, upper_bounds):
    (out, nl, nu), _ = _run(x, lower_bounds, upper_bounds)
    return (out, nl, nu)


# revision 18
# speedup vs baseline: 1.5895x; 1.0389x over previous
"""DeepPoly SPU transformer — Trainium2 Bass kernel (custom-DVE edition).

Elementwise over N=16777216; sharded across 8 NeuronCores (2M elems each,
viewed as [nt x 128 x fd] fp16).  All wire traffic is fp16 (24MB/core round
trip); the three input streams are packed into one DRAM tensor (and the
three outputs into another) so each chunk needs one input DMA + one output
DMA.  The whole per-element DAG is collapsed into 3 fused custom-DVE ops +
2 ACT transcendentals + a small ACT/Pool tail, so every engine sits below
the DMA roofline (~8.7us per 128x2048 chunk) and the kernel is purely
DMA-bound.

Math (per element; Z = sqrt(0.5), spu(t) = t^2-0.5 for t>=0 else
sigmoid(-t)-1).  Device emits nl/nu in doubled space (host applies the
constant affine epilogue  nl = 0.5*nlD - 0.5,  nu = 0.5*nuD):

  sld = tanh(-l/2) = 2*(sigmoid(-l) - 0.5)      [ACT; sign(sld) = -sign(l)]
  sgx = sigmoid(x)                              [ACT]
  out = relu(x)^2 - min(sgx, 0.5)               [ACT relu/square + Pool]
  P2  = 2*g2m*(l - g2m/4),  g2m = 2Z*max([l<0], (u+l)/(2Z))   [custom DVE:
        G1 = max([l<0], (u+l)/(2Z));  P2 = G1*(4Z*l - G1)]
  SD  = sld*[u<=0] - BIG*([u>=Z] + [sld<=0])    [custom DVE]
  nuD = max(2*relu(u)^2, sld) + (2*[sld<=0]-1)  [custom DVE]
  nlD = max(P2, SD)                             [DVE tt max]

Case boundaries (l vs 0, u vs 0, u vs Z) are pinned to the f32 side during
host-side fp16 conversion; l<0 is additionally kept <= -2.5e-7 so that
tanh(-l/2) cannot round to +-0 in fp16 (sld's sign carries [l<0] on
device).  Validated vs the f64 reference: relmax_vs_scale ~ 1e-3 on all
three outputs (tolerance 2e-2).
"""

import numpy as np

import concourse.bass as bass
import concourse.bacc as bacc
import concourse.mybir as mybir
from concourse.tile import TileContext
from concourse.bass_utils import run_bass_kernel_spmd
import concourse.dve_ops as dve_ops
from concourse.dve_spec import (
    Spec, Src0, Src1, C0, C1, Zero, One, maxx, minn, relu, sq, lower as _dve_lower,
    _has_src1,
)
from concourse.dve_uop import DveOpSpec

_N = 16777216
_NCORES = 8
_P = 128
_FDT = _N // _NCORES // _P  # 16384 free elems per partition per core

_Z32 = np.float32(np.sqrt(0.5))
_Z16 = float(np.float16(np.sqrt(0.5)))
_INV2Z = float(np.float32(1.0 / (2.0 * float(_Z32))))
_FOURZ = float(np.float32(4.0 * float(_Z32)))
_BIG = 1000.0

_AF = mybir.ActivationFunctionType
_OP = mybir.AluOpType
_F16 = mybir.dt.float16


# --------------------------------------------------------------------------
# custom DVE ops (registered once per process via the documented extension
# point in concourse.dve_ops; names are namespaced to this kernel)
# --------------------------------------------------------------------------

def _register(name, spec):
    if name in dve_ops._SUB_OPCODE_FOR_NAME:
        for op in dve_ops.OPS:
            if op.name == name:
                return op
    row = max(dve_ops._SUB_OPCODE_FOR_NAME.values()) + 1
    assert row < 0x20, "custom-DVE opcode rows exhausted"
    shas = {}
    for ver in ("v3", "v4"):
        u = _dve_lower(spec, ver=ver)
        shas[ver] = DveOpSpec(
            name=name, opcode=row, uops=u, rd1_en=_has_src1(spec)
        ).sha(ver)
    op = dve_ops.DveOp(name, spec, subdim=False, uops_sha=shas)
    dve_ops.OPS.append(op)
    dve_ops._SUB_OPCODE_FOR_NAME[name] = row
    dve_ops.CUSTOM_DVE_SPECS[name] = spec
    return op


def _ref_p2(in0, in1, c0, c1, c2):
    l = in0.astype(np.float32)
    u = in1.astype(np.float32)
    s2s = (l + u) * np.float32(c0)
    g1 = np.maximum((l < 0).astype(np.float32), s2s)
    return g1 * (l * np.float32(c1) - g1)


_g1 = maxx(Src0 < Zero, (Src0 + Src1) * C0)
_SPEC_P2 = Spec(body=_g1 * (Src0 * C1 - _g1), reference=_ref_p2)


def _ref_sd(in0, in1, c0, c1, c2):
    u = in0.astype(np.float32)
    sld = in1.astype(np.float32)
    m1 = (u <= 0).astype(np.float32)
    m2 = (u >= np.float32(c0)).astype(np.float32)
    m3 = (sld <= 0).astype(np.float32)
    return sld * m1 + (m2 + m3) * np.float32(c1)


_SPEC_SD = Spec(
    body=Src1 * (Src0 <= Zero) + ((Src0 >= C0) + (Src1 <= Zero)) * C1,
    reference=_ref_sd,
)


def _ref_nu(in0, in1, c0, c1, c2):
    u = in0.astype(np.float32)
    sld = in1.astype(np.float32)
    r2 = np.maximum(u, 0) ** 2
    c = (sld <= 0).astype(np.float32)
    return np.maximum(r2 + r2, sld) + (c + c - 1.0)


_r2 = sq(relu(Src0))
_c = (Src1 <= Zero)
_SPEC_NU = Spec(body=maxx(_r2 + _r2, Src1) + (_c + _c - One), reference=_ref_nu)


def _ref_out(in0, in1, c0, c1, c2):
    x = in0.astype(np.float32)
    sgx = in1.astype(np.float32)
    return np.maximum(x, 0) ** 2 - np.minimum(sgx, np.float32(c0))


_SPEC_OUT = Spec(body=sq(relu(Src0)) - minn(Src1, C0), reference=_ref_out)

_OP_P2 = _register("SPU_P2_ANT", _SPEC_P2)
_OP_SD = _register("SPU_SD_ANT", _SPEC_SD)
_OP_NU = _register("SPU_NU_ANT", _SPEC_NU)
_OP_OUT = _register("SPU_OUT_ANT", _SPEC_OUT)


# --------------------------------------------------------------------------
# kernel build
# --------------------------------------------------------------------------

def _build_nc(fd=2048, io_bufs=4, out_bufs=4, tmp_bufs=3, ramp="s24e2",
              dve_head=2, dve_tail=0, rx_dve=0, in_q="sp", out_q="sp",
              emit_order="dma_first", skew=3, hp_out=0, split_out=1, fdt=_FDT):
    """dve_head/dve_tail: how many chunks at each end of the pipeline route
    their `out` stream through the fused DVE op (low latency) instead of the
    ACT/Pool tail (better steady-state balance).  rx_dve: chunks whose
    relu(x) runs as a DVE tensor_scalar instead of ACT."""
    nc = bacc.Bacc(trn_type="TRN2", debug=False, num_devices=_NCORES)
    nt = fdt // fd

    def _q(name):
        return {"sp": nc.sync, "act": nc.scalar, "dve": nc.vector,
                "pool": nc.gpsimd}[name]

    t_in = nc.dram_tensor("pin", [nt, _P, 3, fd], _F16, kind="ExternalInput")
    t_out = nc.dram_tensor("pout", [nt, _P, 3, fd], _F16, kind="ExternalOutput")

    with TileContext(nc) as tc:
        with tc.tile_pool(name="io", bufs=io_bufs) as iop, \
             tc.tile_pool(name="ot", bufs=out_bufs) as otp, \
             tc.tile_pool(name="tmp", bufs=tmp_bufs) as tp:

            if ramp == "end":
                chunks = [(i, 0, fd) for i in range(nt - 1)]
                chunks += [(nt - 1, c, fd // 2) for c in range(0, fd, fd // 2)]
            elif ramp == "end4":
                chunks = [(i, 0, fd) for i in range(nt - 1)]
                chunks += [(nt - 1, c, fd // 4) for c in range(0, fd, fd // 4)]
            elif ramp == "both":
                chunks = [(0, c, fd // 2) for c in range(0, fd, fd // 2)]
                chunks += [(i, 0, fd) for i in range(1, nt - 1)]
                chunks += [(nt - 1, c, fd // 4) for c in range(0, fd, fd // 4)]
            elif ramp == "s4e2":
                chunks = [(0, c, fd // 4) for c in range(0, fd, fd // 4)]
                chunks += [(i, 0, fd) for i in range(1, nt - 1)]
                chunks += [(nt - 1, c, fd // 2) for c in range(0, fd, fd // 2)]
            elif ramp == "s4e4":
                chunks = [(0, c, fd // 4) for c in range(0, fd, fd // 4)]
                chunks += [(i, 0, fd) for i in range(1, nt - 1)]
                chunks += [(nt - 1, c, fd // 4) for c in range(0, fd, fd // 4)]
            elif ramp == "s2e2":
                chunks = [(0, c, fd // 2) for c in range(0, fd, fd // 2)]
                chunks += [(i, 0, fd) for i in range(1, nt - 1)]
                chunks += [(nt - 1, c, fd // 2) for c in range(0, fd, fd // 2)]
            elif ramp == "s24e2":
                chunks = [(0, 0, fd // 4), (0, fd // 4, fd // 4), (0, fd // 2, fd // 2)]
                chunks += [(i, 0, fd) for i in range(1, nt - 1)]
                chunks += [(nt - 1, c, fd // 2) for c in range(0, fd, fd // 2)]
            elif ramp == "s24e24":
                chunks = [(0, 0, fd // 4), (0, fd // 4, fd // 4), (0, fd // 2, fd // 2)]
                chunks += [(i, 0, fd) for i in range(1, nt - 1)]
                chunks += [(nt - 1, 0, fd // 2), (nt - 1, fd // 2, fd // 4),
                           (nt - 1, 3 * fd // 4, fd // 4)]
            elif ramp == "s24":
                chunks = [(0, 0, fd // 4), (0, fd // 4, fd // 4), (0, fd // 2, fd // 2)]
                chunks += [(i, 0, fd) for i in range(1, nt)]
            elif ramp == "s24e2x":
                chunks = [(0, 0, fd // 4), (0, fd // 4, fd // 4), (0, fd // 2, fd // 2)]
                chunks += [(i, 0, fd) for i in range(1, nt - 2)]
                chunks += [(i, c, fd // 2) for i in (nt - 2, nt - 1)
                           for c in range(0, fd, fd // 2)]
            elif ramp == "s8e2":
                chunks = [(0, c, fd // 8) for c in range(0, fd, fd // 8)]
                chunks += [(i, 0, fd) for i in range(1, nt - 1)]
                chunks += [(nt - 1, c, fd // 2) for c in range(0, fd, fd // 2)]
            else:
                chunks = [(i, 0, fd) for i in range(nt)]

            n = len(chunks)
            st = [None] * n

            def SDMA(ci):
                i, c0, fdc = chunks[ci]
                with tc.high_priority():
                    it = iop.tile([_P, 3 * fdc], _F16, tag="in")
                    _q(in_q).dma_start(
                        out=it[:].rearrange("p (s f) -> p s f", s=3),
                        in_=t_in[i, :, :, c0:c0 + fdc])
                st[ci] = dict(it=it)

            def S0(ci):
                i, c0, fdc = chunks[ci]
                it = st[ci]["it"]
                l = it[:, 0:fdc]
                x = it[:, 2 * fdc:3 * fdc]
                sld = tp.tile([_P, fdc], _F16, tag="sld")
                sgx = tp.tile([_P, fdc], _F16, tag="sgx")
                nc.scalar.activation(sld[:], l, _AF.Tanh, scale=-0.5)
                nc.scalar.activation(sgx[:], x, _AF.Sigmoid)
                dve_out = ci < dve_head or ci >= n - dve_tail
                if not dve_out:
                    rx2 = tp.tile([_P, fdc], _F16, tag="rx2")
                    if rx_dve and (ci % rx_dve == 0):
                        rx = tp.tile([_P, fdc], _F16, tag="rx")
                        nc.vector.tensor_scalar(rx[:], x, 0.0, None, _OP.max)
                        nc.scalar.activation(rx2[:], rx[:], _AF.Square)
                    else:
                        nc.scalar.activation(rx2[:], x, _AF.Relu)
                        nc.scalar.activation(rx2[:], rx2[:], _AF.Square)
                    st[ci].update(rx2=rx2)
                st[ci].update(sld=sld, sgx=sgx, dve_out=dve_out)

            def S1(ci):
                i, c0, fdc = chunks[ci]
                d = st[ci]
                it = d["it"]
                l3 = it[:, 0:fdc].unsqueeze(1)
                u3 = it[:, fdc:2 * fdc].unsqueeze(1)
                sld3 = d["sld"][:].unsqueeze(1)
                ot = otp.tile([_P, 3 * fdc], _F16, tag="out")
                p2 = tp.tile([_P, fdc], _F16, tag="p2")
                sd = tp.tile([_P, fdc], _F16, tag="sd")
                nc.vector._custom_dve(_OP_P2, out=p2[:].unsqueeze(1), in0=l3,
                                      in1=u3, s0=_INV2Z, s1=_FOURZ)
                nc.vector._custom_dve(_OP_SD, out=sd[:].unsqueeze(1), in0=u3,
                                      in1=sld3, s0=_Z16, s1=-_BIG)
                nc.vector._custom_dve(_OP_NU, out=ot[:, 2 * fdc:3 * fdc].unsqueeze(1),
                                      in0=u3, in1=sld3)
                if d["dve_out"]:
                    x3 = it[:, 2 * fdc:3 * fdc].unsqueeze(1)
                    nc.vector._custom_dve(_OP_OUT, out=ot[:, 0:fdc].unsqueeze(1),
                                          in0=x3, in1=d["sgx"][:].unsqueeze(1),
                                          s0=0.5)
                else:
                    sg = tp.tile([_P, fdc], _F16, tag="sg")
                    nc.gpsimd.tensor_scalar(sg[:], d["sgx"][:], 0.5, None, _OP.min)
                    st[ci].update(sg=sg)
                st[ci].update(ot=ot, p2=p2, sd=sd)

            def S2(ci):
                i, c0, fdc = chunks[ci]
                d = st[ci]
                ot = d["ot"]
                if not d["dve_out"]:
                    nc.gpsimd.tensor_tensor(ot[:, 0:fdc], d["rx2"][:], d["sg"][:],
                                            _OP.subtract)
                nc.vector.tensor_tensor(ot[:, fdc:2 * fdc], d["p2"][:], d["sd"][:],
                                        _OP.max)
                if split_out and not d["dve_out"]:
                    otv = ot[:].rearrange("p (s f) -> p s f", s=3)
                    _q(out_q).dma_start(
                        out=t_out[i, :, 1:3, c0:c0 + fdc],
                        in_=otv[:, 1:3, :])
                    _q(out_q).dma_start(
                        out=t_out[i, :, 0:1, c0:c0 + fdc],
                        in_=otv[:, 0:1, :])
                elif hp_out:
                    with tc.high_priority():
                        _q(out_q).dma_start(
                            out=t_out[i, :, :, c0:c0 + fdc],
                            in_=ot[:].rearrange("p (s f) -> p s f", s=3))
                else:
                    _q(out_q).dma_start(
                        out=t_out[i, :, :, c0:c0 + fdc],
                        in_=ot[:].rearrange("p (s f) -> p s f", s=3))
                st[ci] = None

            if skew == 0:
                for ci in range(n):
                    SDMA(ci); S0(ci); S1(ci); S2(ci)
            elif skew == 1:
                for k in range(n + 1):
                    if k < n: SDMA(k)
                    if 0 <= k - 1 < n:
                        S0(k - 1); S1(k - 1); S2(k - 1)
            elif skew == 2:
                for k in range(n + 2):
                    if k < n: SDMA(k)
                    if 0 <= k - 1 < n: S0(k - 1)
                    if 0 <= k - 2 < n: S1(k - 2); S2(k - 2)
            else:
                order = {
                    "dma_first": lambda k: [(SDMA, k), (S0, k - 1), (S1, k - 2), (S2, k - 3)],
                    "s2_mid": lambda k: [(SDMA, k), (S0, k - 1), (S2, k - 3), (S1, k - 2)],
                }[emit_order]
                for k in range(n + 3):
                    for fn, ci in order(k):
                        if 0 <= ci < n:
                            fn(ci)

    nc.compile()
    return nc


_NC_CACHE = {}


def _get_nc(**kw):
    key = tuple(sorted(kw.items()))
    if key not in _NC_CACHE:
        _NC_CACHE[key] = _build_nc(**kw)
    return _NC_CACHE[key]


def _prep_inputs(x, lower_bounds, upper_bounds):
    """fp16 conversion with case-boundary pinning (see module docstring)."""
    F16 = np.float16
    x16 = x.astype(F16)
    l16 = lower_bounds.astype(F16)
    u16 = upper_bounds.astype(F16)
    # l<0 must stay strictly negative AND large enough that tanh(-l/2)
    # cannot round to zero in fp16 (sld's sign carries [l<0] on device).
    l16 = np.where((lower_bounds < 0) & (l16 >= -2.5e-7), F16(-2.5e-7), l16)
    # u>0 must stay strictly positive (case selection uses u<=0).
    u16 = np.where((upper_bounds > 0) & (u16 <= 0), F16(6e-8), u16)
    # u vs Z: the reference jumps at u==Z; keep each element on its f32 side.
    z16 = F16(_Z16)
    below = np.nextafter(z16, F16(0))
    u16 = np.where((upper_bounds >= _Z32) & (u16 < z16), z16, u16)
    u16 = np.where((upper_bounds < _Z32) & (u16 >= z16), below, u16)
    return x16, l16, u16


def _run(x, lower_bounds, upper_bounds, trace=False, **build_kw):
    assert x.shape == (_N,) and x.dtype == np.float32
    nc = _get_nc(**build_kw)
    fd = build_kw.get("fd", 2048)
    nt = _FDT // fd
    x16, l16, u16 = _prep_inputs(x, lower_bounds, upper_bounds)
    shp = (_NCORES, nt, _P, fd)
    packed = np.empty((_NCORES, nt, _P, 3, fd), dtype=np.float16)
    packed[..., 0, :] = l16.reshape(shp)
    packed[..., 1, :] = u16.reshape(shp)
    packed[..., 2, :] = x16.reshape(shp)
    in_maps = [{"pin": packed[c]} for c in range(_NCORES)]
    res = run_bass_kernel_spmd(
        nc, in_maps, core_ids=list(range(_NCORES)), trace=trace
    )
    pout = np.stack([res.results[c]["pout"] for c in range(_NCORES)])
    out = np.ascontiguousarray(pout[..., 0, :]).reshape(-1).astype(np.float32)
    nl = np.ascontiguousarray(pout[..., 1, :]).reshape(-1).astype(np.float32)
    nl = nl * 0.5 - 0.5  # device emits nl in doubled +0.5 space
    nu = np.ascontiguousarray(pout[..., 2, :]).reshape(-1).astype(np.float32)
    nu = nu * 0.5  # device emits nu in doubled space
    return (out, nl, nu), res


def kernel(x, lower_bounds, upper_bounds):
    (out, nl, nu), _ = _run(x, lower_bounds, upper_bounds)
    return (out, nl, nu)
